# revision 26
# baseline (speedup 1.0000x reference)
"""Multi-head attention Bass kernel for Trainium2, SPMD over 8 NeuronCores.

Problem: B=4, S=2048, D=1024, 16 heads x 64. Sharding: core = (batch b, head-group hg)
with b in 0..3, hg in 0..1 -> each core computes 8 heads of one batch.

The PE array is treated as a 4x4 grid of 32x32 sub-arrays; concurrent
matmuls with disjoint (row-group, col-group) rectangles overlap (~one
N=512 "slot" of ~306ns for the whole pack). Per kc iteration (128 k
positions, q block 512 wide):

  - scores pack (1 slot): the two heads' S^T matmuls (K=64 contraction)
    row-pack into opposite halves of the array, writing the two banks of
    one [128, 2(head), 512] PSUM tile (double buffered).
  - one 1024-col exp on ScalarE covers both heads (scale folded in),
    bf16 out; ScalarE is the target pacing engine (~1.05us/kc).
  - AV pack (1 slot): O^T[h0] (cols 0-63) and O^T[h1] (cols 64-127)
    col-pack into ONE shared PSUM bank; kc=0 uses start=True on the first
    tile only (clears the bank) and start=False on the second (overwrite-
    where-unset per has_written semantics).
  - denominator pack (0.5 slot): every second kc, four M=1 col-tiles
    (ones[128,1] lhsT at col positions 0/32/64/96) accumulate
    sum_k exp for (kc-1,kc)x(h0,h1) into one [128,512] PSUM bank; the
    finalize step adds the two partial rows per head.
  - projections for later head pairs are emitted as single-matmul filler
    closures interleaved into the loop.

PSUM budget (8 banks): scores 2x[128,2,512]=4, O accum [128,512]x2bufs=2,
den [128,512]=1, projection staging [128,512]=1.
"""
import numpy as np
import ml_dtypes
from contextlib import ExitStack

import concourse.tile as tile
import concourse.mybir as mybir
from concourse import bacc
from concourse.bass_utils import run_bass_kernel_spmd

P = 128
DH = 64
BF = mybir.dt.bfloat16
F32 = mybir.dt.float32


def build_attention(S=2048, D=1024, HPC=8, loop_n=1, ablate=(), pbufs=4, pops=2,
                    early_proj=False):
    """Build the per-core SPMD program. HPC = heads per core (even).

    loop_n > 1 wraps the whole body in a hardware loop (for timing)."""
    DC = D // P        # D chunks of 128
    KC = S // P        # k chunks of 128
    NQ = S // 512      # q blocks of 512
    HP = HPC // 2      # head pairs
    CW = HPC * DH      # core output width
    SCALE = 1.0 / float(np.sqrt(DH))

    nc = bacc.Bacc("TRN2")
    xq = nc.dram_tensor("xq", [DC, P, S], BF, kind="ExternalInput")
    xk = nc.dram_tensor("xk", [DC, P, S], BF, kind="ExternalInput")
    xv = nc.dram_tensor("xv", [DC, P, S], BF, kind="ExternalInput")
    wq = nc.dram_tensor("wq", [DC, P, CW], BF, kind="ExternalInput")
    wk = nc.dram_tensor("wk", [DC, P, CW], BF, kind="ExternalInput")
    wv = nc.dram_tensor("wv", [DC, P, CW], BF, kind="ExternalInput")
    out = nc.dram_tensor("out", [HPC, DH, S], F32, kind="ExternalOutput")

    with tile.TileContext(nc) as tc, ExitStack() as ctx:
        xpool = ctx.enter_context(tc.tile_pool(name="x", bufs=1))
        wpool = ctx.enter_context(tc.tile_pool(name="w", bufs=1))
        vpool = ctx.enter_context(tc.tile_pool(name="v", bufs=1))
        qkpool = ctx.enter_context(
            tc.tile_pool(name="qk", bufs=4 if early_proj else 2))
        ppool = ctx.enter_context(tc.tile_pool(name="p", bufs=pbufs))
        opool = ctx.enter_context(tc.tile_pool(name="one", bufs=1))
        ostag = ctx.enter_context(tc.tile_pool(name="ost", bufs=3))
        outp = ctx.enter_context(tc.tile_pool(name="outp", bufs=3))
        rpool = ctx.enter_context(tc.tile_pool(name="r", bufs=2))
        ps_s = ctx.enter_context(tc.tile_pool(name="ps_s", bufs=2, space="PSUM"))
        ps_o = ctx.enter_context(tc.tile_pool(name="ps_o", bufs=2, space="PSUM"))
        ps_d = ctx.enter_context(tc.tile_pool(name="ps_d", bufs=1, space="PSUM"))
        ps_m = ctx.enter_context(tc.tile_pool(name="ps_m", bufs=1, space="PSUM"))

        xs, ws = {}, {}
        vt = None
        ones = None

        def emit_loads():
            nonlocal vt, ones
            for name, dram in [("q", wq), ("k", wk), ("v", wv)]:
                t = wpool.tile([P, DC, CW], BF, tag="w" + name, name="w" + name)
                for dc in range(DC):
                    nc.sync.dma_start(t[:, dc, :], dram[dc])
                ws[name] = t
            for name, dram in [("q", xq), ("k", xk), ("v", xv)]:
                t = xpool.tile([P, DC, S], BF, tag="x" + name, name="x" + name)
                for dc in range(DC):
                    nc.sync.dma_start(t[:, dc, :], dram[dc])
                xs[name] = t
            vt = vpool.tile([P, KC, HPC, DH], BF, tag="V", name="vt")
            ones = opool.tile([P, 1], BF, tag="ones", name="ones")
            nc.any.memset(ones[:], 1.0)

        def proj_v_kc(kc):
            pv = ps_m.tile([P, 512], F32, tag="proj", name="pv")[:, :CW]
            for dc in range(DC):
                nc.tensor.matmul(
                    pv,
                    xs["v"][:, dc, kc * P : (kc + 1) * P],
                    ws["v"][:, dc, :],
                    start=(dc == 0),
                    stop=(dc == DC - 1),
                )
            nc.vector.tensor_copy(
                vt[:, kc, :, :],
                pv.rearrange("p (h d) -> p h d", d=DH),
            )

        def proj_qk_chunk(t, which, hp, qb):
            pp = ps_m.tile([P, 512], F32, tag="proj")
            for dc in range(DC):
                nc.tensor.matmul(
                    pp[:],
                    ws[which][:, dc, hp * P : (hp + 1) * P],
                    xs[which][:, dc, qb * 512 : (qb + 1) * 512],
                    start=(dc == 0),
                    stop=(dc == DC - 1),
                )
            nc.vector.tensor_copy(t[:, qb * 512 : (qb + 1) * 512], pp[:])

        def new_qk(which):
            return qkpool.tile([P, S], BF, tag=which, name=which + "t")

        def proj_qk_fillers(t, which, hp):
            """Projection of one tensor for head pair hp as a list of small
            filler closures (one matmul each; the last also evacuates)."""
            fillers = []
            for qb in range(NQ):
                state = {}

                def mk(dc, qb=qb, state=state):
                    def f():
                        if dc == 0:
                            state["pp"] = ps_m.tile([P, 512], F32, tag="proj",
                                                    name="pp")
                        pp = state["pp"]
                        nc.tensor.matmul(
                            pp[:],
                            ws[which][:, dc, hp * P : (hp + 1) * P],
                            xs[which][:, dc, qb * 512 : (qb + 1) * 512],
                            start=(dc == 0),
                            stop=(dc == DC - 1),
                        )
                        if dc == DC - 1:
                            nc.vector.tensor_copy(
                                t[:, qb * 512 : (qb + 1) * 512], pp[:])
                    return f

                fillers += [mk(d) for d in range(DC)]
            return fillers

        def finalize_fillers(osb, den_sb, hp, qb):
            """Normalize + store one finished q block: one closure per
            (head, op). den_sb holds the 4 denominator partial rows."""
            fillers = []
            if "nofin" in ablate:
                return []
            for h in (0, 1):
                ch = hp * 2 + h
                state = {}

                def mv(h=h, state=state, den_sb=den_sb):
                    # den_sb rows: 0=(even kc,h0), 32=(even kc,h1),
                    #              64=(odd kc,h0), 96=(odd kc,h1).
                    # DMA the two partial rows for head h to partition 0.
                    dA = rpool.tile([1, 512], F32, tag="dA", name="dA")
                    dB = rpool.tile([1, 512], F32, tag="dB", name="dB")
                    nc.sync.dma_start(dA[0:1, :], den_sb[h * 32 : h * 32 + 1, :])
                    nc.sync.dma_start(
                        dB[0:1, :], den_sb[64 + h * 32 : 64 + h * 32 + 1, :])
                    state["dA"], state["dB"] = dA, dB

                def rec(state=state):
                    dsum = rpool.tile([1, 512], F32, tag="ds", name="dsum")
                    nc.vector.tensor_tensor(
                        dsum[:], state["dA"][:], state["dB"][:],
                        mybir.AluOpType.add)
                    rsb = rpool.tile([1, 512], F32, tag="rc", name="rsb")
                    nc.vector.reciprocal(rsb[:], dsum[:])
                    state["rsb"] = rsb

                def bc(state=state):
                    # broadcast to all 128 partitions so the head-h slice is
                    # partition-aligned with osb's slice
                    rbc = rpool.tile([P, 512], F32, tag="rbc", name="rbc")
                    nc.gpsimd.partition_broadcast(rbc[:], state["rsb"][0:1, :])
                    state["rbc"] = rbc

                def norm(ch=ch, qb=qb, h=h, osb=osb, state=state):
                    ot = outp.tile([P, 512], F32, tag="ot", name="ot")
                    sl = slice(h * DH, (h + 1) * DH)
                    nc.vector.tensor_tensor(
                        ot[sl, :], osb[sl, :], state["rbc"][sl, :],
                        mybir.AluOpType.mult)
                    nc.sync.dma_start(
                        out[ch, :, qb * 512 : (qb + 1) * 512], ot[sl, :])

                fillers += [mv, rec, bc, norm]
            return fillers

        def attn_block(hp, qb, qt, kt, first, proj_q, fin_q):
            """Attention for head pair hp, q block qb (512 wide)."""
            while len(fin_q) > 8:
                fin_q.pop(0)()
            o_ps = ps_o.tile([P, 512], F32, tag="O", name="o_ps")
            den_ps = ps_d.tile([P, 512], F32, tag="den", name="den_ps")

            def emit_scores(kc):
                s = ps_s.tile([P, 2, 512], F32, tag="S", name="s")
                for h in (0, 1):
                    # K=64 contraction; the two heads' lhsT/rhs live on
                    # opposite partition halves -> row-packed in the PE array
                    nc.tensor.matmul(
                        s[:, h, :],
                        kt[h * DH : (h + 1) * DH, kc * P : (kc + 1) * P],
                        qt[h * DH : (h + 1) * DH, qb * 512 : (qb + 1) * 512],
                        start=True,
                        stop=True,
                    )
                return s

            s_cur = emit_scores(0)
            pts = {}
            for kc in range(KC):
                pt = ppool.tile([P, 2, 512], BF, tag="pt")
                pts[kc] = pt
                nc.scalar.activation(
                    pt[:], s_cur[:], mybir.ActivationFunctionType.Exp,
                    scale=SCALE)
                if kc + 1 < KC:
                    s_cur = emit_scores(kc + 1)
                # V projection just-in-time during the first attn pass
                if first and qb == 0:
                    proj_v_kc(kc)
                # AV pack: both heads col-packed into one bank
                for h in (0, 1):
                    ch = hp * 2 + h
                    nc.tensor.matmul(
                        o_ps[h * DH : (h + 1) * DH, :],
                        vt[:, kc, ch, :],
                        pt[:, h, :],
                        start=(kc == 0),
                        stop=(kc == KC - 1),
                        skip_group_check=(h == 1),
                    )
                # denominator pack: 4 M=1 col-tiles for (kc-1, kc) x (h0, h1)
                if kc % 2 == 1:
                    for j, (kcd, h) in enumerate(
                            [(kc - 1, 0), (kc - 1, 1), (kc, 0), (kc, 1)]):
                        nc.tensor.matmul(
                            den_ps[j * 32 : j * 32 + 1, :],
                            ones[:],
                            pts[kcd][:, h, :],
                            start=(kc == 1),
                            stop=(kc == KC - 1),
                            tile_position=(0, j * 32),
                            skip_group_check=(j > 0),
                        )
                    if kc - 2 in pts:
                        del pts[kc - 2], pts[kc - 3]
                # interleave deferred work while ScalarE paces the loop
                if not (first and qb == 0) and kc < KC - 1:
                    budget = pops
                    while budget and (proj_q or fin_q):
                        (proj_q or fin_q).pop(0)()
                        budget -= 1
            # evacuate O and den PSUM now; normalize runs as fillers later
            osb = ostag.tile([P, 512], F32, tag="osb")
            nc.vector.tensor_copy(osb[:], o_ps[:])
            den_sb = ostag.tile([97, 512], F32, tag="densb")
            for j in range(4):
                nc.vector.tensor_copy(
                    den_sb[j * 32 : j * 32 + 1, :], den_ps[j * 32 : j * 32 + 1, :])
            return osb, den_sb

        def emit_body():
            emit_loads()
            qt = new_qk("q")
            kt = new_qk("k")
            for qb in range(NQ):
                proj_qk_chunk(qt, "q", 0, qb)
                proj_qk_chunk(kt, "k", 0, qb)
            proj_q, fin_q = [], []
            nxt = {}
            for hp in range(HP):
                if early_proj:
                    if hp == 0:
                        for h2 in range(1, HP):
                            qt2, kt2 = new_qk("q"), new_qk("k")
                            nxt[h2] = (qt2, kt2)
                            proj_q += proj_qk_fillers(qt2, "q", h2)
                            proj_q += proj_qk_fillers(kt2, "k", h2)
                elif hp + 1 < HP:
                    qt_next = new_qk("q")
                    kt_next = new_qk("k")
                    nxt[hp + 1] = (qt_next, kt_next)
                    proj_q += proj_qk_fillers(qt_next, "q", hp + 1)
                    proj_q += proj_qk_fillers(kt_next, "k", hp + 1)
                for qb in range(NQ):
                    osb, den_sb = attn_block(hp, qb, qt, kt, first=(hp == 0),
                                             proj_q=proj_q, fin_q=fin_q)
                    fin_q += finalize_fillers(osb, den_sb, hp, qb)
                if not early_proj:
                    # the next head pair's projections must be fully emitted
                    # before its attention reads them
                    for f in proj_q:
                        f()
                    proj_q = []
                elif hp + 1 < HP:
                    # ensure the next pair's projections are fully emitted
                    while proj_q and len(proj_q) > (HP - 2 - hp) * 2 * NQ * DC:
                        proj_q.pop(0)()
                if hp + 1 < HP:
                    qt, kt = nxt[hp + 1]
            for f in fin_q:
                f()

        if loop_n > 1:
            with tc.For_i(0, loop_n, 1):
                emit_body()
        else:
            emit_body()

    nc.compile()
    return nc


_NC_CACHE = {}


def _get_nc(S, D, HPC):
    key = (S, D, HPC)
    if key not in _NC_CACHE:
        _NC_CACHE[key] = build_attention(S, D, HPC)
    return _NC_CACHE[key]


def _prep_core_inputs(q_seq, k_seq, v_seq, WQ, WK, WV, b, hg, HPC, D):
    """Host-side shard prep for core (batch b, head group hg)."""
    DC = D // P
    CW = HPC * DH
    bf16 = ml_dtypes.bfloat16

    def xt(x):  # [S, D] -> [DC, P, S] (D-major transpose)
        return np.ascontiguousarray(x.T.reshape(DC, P, -1)).astype(bf16)

    def wslice(w):  # [D, out] -> [DC, P, CW]
        return np.ascontiguousarray(
            w[:, hg * CW : (hg + 1) * CW].reshape(DC, P, CW)
        ).astype(bf16)

    return {
        "xq": xt(q_seq[b]),
        "xk": xt(k_seq[b]),
        "xv": xt(v_seq[b]),
        "wq": wslice(WQ),
        "wk": wslice(WK),
        "wv": wslice(WV),
    }


def kernel(q_seq, k_seq, v_seq, WQ, WK, WV, _trace=False):
    q_seq = np.asarray(q_seq, dtype=np.float32)
    k_seq = np.asarray(k_seq, dtype=np.float32)
    v_seq = np.asarray(v_seq, dtype=np.float32)
    WQ = np.asarray(WQ, dtype=np.float32)
    WK = np.asarray(WK, dtype=np.float32)
    WV = np.asarray(WV, dtype=np.float32)

    B, S, D = q_seq.shape
    NB_HEAD = WQ.shape[1] // DH
    n_cores = 8
    groups_per_batch = n_cores // B          # 2 head groups
    HPC = NB_HEAD // groups_per_batch        # 8 heads per core
    CW = HPC * DH

    nc = _get_nc(S, D, HPC)

    in_maps = []
    for core in range(n_cores):
        b, hg = core // groups_per_batch, core % groups_per_batch
        in_maps.append(_prep_core_inputs(q_seq, k_seq, v_seq, WQ, WK, WV, b, hg, HPC, D))

    res = run_bass_kernel_spmd(
        nc, in_maps, core_ids=list(range(n_cores)), trace=_trace,
        **({"trace_cores": [0], } if _trace else {}),
    )
    if _trace:
        print(f"HW exec time: {res.exec_time_ns} ns")
        if res.instructions_and_trace:
            print("trace:", res.instructions_and_trace[1])

    out = np.empty((B, S, NB_HEAD * DH), dtype=np.float32)
    for core in range(n_cores):
        b, hg = core // groups_per_batch, core % groups_per_batch
        # device output is O^T per head: [HPC, DH, S] -> [S, HPC*DH]
        ot = res.results[core]["out"]
        out[b, :, hg * CW : (hg + 1) * CW] = (
            ot.transpose(2, 0, 1).reshape(S, CW)
        )
    return out
